# revision 14
# baseline (speedup 1.0000x reference)
"""GQA attention kernel for Trainium2, tuned for the axon-tunnel regime.

The dominant cost in this environment is host<->device transfer over the
axon tunnel (~35-60 MiB/s shared across cores), not on-device compute
(~2 ms). So the layout minimizes wire bytes:

  - ONE core computes both batches and ALL 16 heads -> zero duplication of
    x or weights across cores, and every output element is produced exactly
    once on-device (no host-summed partial o_proj).
  - bf16 on the wire for x/weights, fp16 for cos/sin and the output.
  - The causal block-masks are generated on-chip (affine_select), not
    uploaded.
  - The donated output buffer is created on-device (jitted zeros) instead
    of being uploaded.
  Total ~21 MiB vs ~190 MiB for the original 8-core partial-sum design.

On-device structure per batch b: q/k/v projections for all 16 heads
(4 kv heads), RoPE, causal flash-style attention per kv-group, and a full
o_proj over all heads, written back as fp16 [T, D].

Device layout tricks:
  - x is shipped pre-transposed (xT [D, T]) so projections need no on-chip
    transpose.
  - Wq/Wk columns are permuted per head to [even dims | odd dims] so RoPE is
    two contiguous [32, .] halves (permutation cancels in q.k).
  - Scores are computed transposed (S^T [tk, tq]); the AV matmul contracts
    over tk on partitions and the softmax denominator comes free from a
    ones-column appended to V (M=65).
  - Each roped kv head is duplicated to partitions 64-127 so the two q-heads
    of an m-tile land on disjoint PE row-groups and run concurrently.
  - Causality at [tk=128, tq=512] block granularity: blocks above the
    diagonal are skipped, diagonal-straddling blocks get a 0/1 mask multiply.
  - Per-head exp scales and the output scale are runtime inputs ([128, .]
    f32 scale tensor, used as per-partition activation scale APs), so int8
    weight/x transport can be re-enabled without recompiling.
"""
import math

import numpy as np
import ml_dtypes

import jax
import jax.numpy as jnp

import concourse.bass as bass
import concourse.bacc as bacc
import concourse.mybir as mybir
import concourse.tile as tile
from concourse.bass import ds, ts
from concourse import bass2jax

B, T, D = 2, 2048, 1024
H, KV, DH = 16, 4, 64
MT = 8                # q m-tiles (2 heads each)
NT = T // 512         # 4 tq tiles of 512
TK = T // 128         # 16 tk tiles of 128
KD = D // 128         # 8 contraction chunks
ROPE_THETA = 500000.0
SCALE = 1.0 / math.sqrt(DH)
N_CORES = 1
NB = B                # batches per core

F32 = mybir.dt.float32
F16 = mybir.dt.float16
BF16 = mybir.dt.bfloat16
I8 = mybir.dt.int8
BF = ml_dtypes.bfloat16

X_INT8 = False  # ship x as int8 (else bf16) -- int8 costs ~1.9% rel err
W_INT8 = False  # ship weights as int8 (else bf16)
SEXP = H + 1          # scale columns per batch: H exp scales + out scale


def _build_body(tc):
    nc = tc.nc
    xdt = I8 if X_INT8 else BF16
    wdt = I8 if W_INT8 else BF16
    xt_d = nc.dram_tensor("xt", [NB * D, T], xdt, kind="ExternalInput")
    wq_d = nc.dram_tensor("wq", [D, H * DH], wdt, kind="ExternalInput")
    wkv_d = nc.dram_tensor("wkv", [D, 2 * KV * DH], wdt, kind="ExternalInput")
    wo_d = nc.dram_tensor("wo", [H * DH, D], wdt, kind="ExternalInput")
    cs_d = nc.dram_tensor("cs", [128, T], F16, kind="ExternalInput")
    scl_d = nc.dram_tensor("scl", [128, NB * SEXP], F32, kind="ExternalInput")
    out_d = nc.dram_tensor("out", [NB * T, D], F16, kind="ExternalOutput")

    def dma_in(dst, src):
        if dst.dtype != src.dtype:
            nc.gpsimd.dma_start(dst, src)   # SWDGE casts during DMA
        else:
            nc.sync.dma_start(dst, src)

    with (
        tc.tile_pool(name="cst", bufs=1) as cst,
        tc.tile_pool(name="pp", bufs=2, space="PSUM") as pp,
        tc.tile_pool(name="sp", bufs=2, space="PSUM") as sp,
        tc.tile_pool(name="avp", bufs=2, space="PSUM") as avp,
        tc.tile_pool(name="bcp", bufs=1, space="PSUM") as bcp,
        tc.tile_pool(name="rtp", bufs=4) as rtp,
        tc.tile_pool(name="esp", bufs=6) as esp,
        tc.tile_pool(name="ysp", bufs=2) as ysp,
        tc.tile_pool(name="rcp", bufs=2) as rcp,
        tc.tile_pool(name="bsp", bufs=2) as bsp,
    ):
        # persistent SBUF tensors (2D: [partitions, flattened blocks])
        xt = cst.tile([128, KD * T], BF16, tag="xt")            # [p, k*T + t]
        wq = cst.tile([128, KD * H * DH], BF16, tag="wq")       # [p, k*1024 + m]
        wkv = cst.tile([128, KD * 2 * KV * DH], BF16, tag="wkv")
        wo = cst.tile([128, KD * D], BF16, tag="wo")            # [p, kc*D + d]
        csb = cst.tile([128, T], F16, tag="csb")                # [cos;sin;cos;sin]
        cs32 = cst.tile([128, T], F32, tag="cs32")
        msk = cst.tile([128, NT * 512], BF16, tag="msk")
        scl = cst.tile([128, NB * SEXP], F32, tag="scl")
        ones = cst.tile([1, 64], BF16, tag="ones")
        qt = cst.tile([128, MT * T], BF16, tag="qt")            # [p, mt*T + t]
        kt = cst.tile([128, KV * T], BF16, tag="kt")            # [p, g*T + t]
        vt = cst.tile([128, KV * TK * (DH + 1)], BF16, tag="vt")
        att = cst.tile([128, MT * T], BF16, tag="att")

        # ---- one-time loads / on-chip tables ----
        for k in range(KD):
            dma_in(wq[:, ds(k * H * DH, H * DH)], wq_d[ds(k * 128, 128), :])
            dma_in(wkv[:, ds(k * 2 * KV * DH, 2 * KV * DH)],
                   wkv_d[ds(k * 128, 128), :])
            dma_in(wo[:, ds(k * D, D)], wo_d[ds(k * 128, 128), :])
        nc.sync.dma_start(csb[:], cs_d[:])
        nc.sync.dma_start(scl[:], scl_d[:])
        nc.vector.tensor_copy(cs32[:], csb[:])
        nc.vector.memset(ones[:], 1.0)
        nc.vector.memset(msk[:], 1.0)
        for d in range(NT):
            # keep 1.0 where key p + 128*d <= query f, else 0
            nc.gpsimd.affine_select(
                out=msk[:, ds(d * 512, 512)], in_=msk[:, ds(d * 512, 512)],
                compare_op=mybir.AluOpType.is_ge, fill=0.0,
                base=-d * 128, pattern=[[1, 512]], channel_multiplier=-1)
        for g in range(KV):
            for j in range(TK):
                nc.vector.memset(
                    vt[:, ds((g * TK + j) * (DH + 1) + DH, 1)], 1.0)

        def rope32(dst, dst_row, dst_col, src, e_row, n):
            """dst rows [dst_row, dst_row+32)+[.. +64) <- roped src halves."""
            e = src[ds(e_row, 32), :]
            o = src[ds(e_row + 32, 32), :]
            c = cs32[ds(e_row, 32), ds(n * 512, 512)]
            s = cs32[ds(e_row + 32, 32), ds(n * 512, 512)]
            t1 = rtp.tile([32, 512], F32, tag="rt")
            t2 = rtp.tile([32, 512], F32, tag="rt")
            nc.vector.tensor_mul(t1[:], e, c)
            nc.vector.tensor_mul(t2[:], o, s)
            nc.vector.tensor_sub(dst[ds(dst_row, 32), ds(dst_col, 512)],
                                 t1[:], t2[:])
            t3 = rtp.tile([32, 512], F32, tag="rt")
            t4 = rtp.tile([32, 512], F32, tag="rt")
            nc.vector.tensor_mul(t3[:], o, c)
            nc.vector.tensor_mul(t4[:], e, s)
            nc.vector.tensor_add(dst[ds(dst_row + 32, 32), ds(dst_col, 512)],
                                 t3[:], t4[:])

        for b in range(NB):
            # x^T for this batch, column-block-major
            for n in range(NT):
                for k in range(KD):
                    dma_in(xt[:, ds(k * T + n * 512, 512)],
                           xt_d[ds(b * D + k * 128, 128), ds(n * 512, 512)])

            # ---- q projection + rope (16 heads = 8 m-tiles of 2) ----
            for mt in range(MT):
                for n in range(NT):
                    ps = pp.tile([128, 512], F32, tag="pp")
                    for k in range(KD):
                        nc.tensor.matmul(
                            ps[:],
                            wq[:, ds(k * H * DH + mt * 128, 128)],
                            xt[:, ds(k * T + n * 512, 512)],
                            start=(k == 0), stop=(k == KD - 1))
                    for h2 in range(2):
                        rope32(qt, h2 * 64, mt * T + n * 512, ps, h2 * 64, n)

            # ---- k projection + rope (4 kv heads, 2 per psum tile) ----
            for kp in range(2):
                for n in range(NT):
                    ps = pp.tile([128, 512], F32, tag="pp")
                    for k in range(KD):
                        nc.tensor.matmul(
                            ps[:],
                            wkv[:, ds(k * 2 * KV * DH + kp * 128, 128)],
                            xt[:, ds(k * T + n * 512, 512)],
                            start=(k == 0), stop=(k == KD - 1))
                    for g2 in range(2):
                        g = kp * 2 + g2
                        rope32(kt, 0, g * T + n * 512, ps, g2 * 64, n)
                        # duplicate to partitions 64-127 for PE row-tiling
                        nc.vector.tensor_copy(
                            kt[ds(64, 64), ds(g * T + n * 512, 512)],
                            kt[ds(0, 64), ds(g * T + n * 512, 512)])

            # ---- v projection (token-major; all 4 kv heads per tile) ----
            for j in range(TK):
                psv = pp.tile([128, 512], F32, tag="pp")
                for k in range(KD):
                    nc.tensor.matmul(
                        psv[:, ds(0, KV * DH)],
                        xt[:, ds(k * T + j * 128, 128)],
                        wkv[:, ds(k * 2 * KV * DH + KV * DH, KV * DH)],
                        start=(k == 0), stop=(k == KD - 1))
                for g in range(KV):
                    nc.scalar.copy(vt[:, ds((g * TK + j) * (DH + 1), DH)],
                                   psv[:, ds(g * DH, DH)])

            # ---- attention: S^T blocks -> exp -> AV with fused denom ----
            for mt in range(MT):
                g = mt // 2
                for i in range(NT):
                    ntk = 4 * (i + 1)
                    av0 = avp.tile([DH + 1, 512], F32, tag="avp")
                    av1 = avp.tile([DH + 1, 512], F32, tag="avp")
                    avs = [av0, av1]
                    for j in range(ntk):
                        for h2 in range(2):
                            sps = sp.tile([128, 512], F32, tag="sp")
                            nc.tensor.matmul(
                                sps[:],
                                kt[ds(h2 * 64, 64), ds(g * T + j * 128, 128)],
                                qt[ds(h2 * 64, 64),
                                   ds(mt * T + i * 512, 512)],
                                start=True, stop=True)
                            es = esp.tile([128, 512], BF16, tag="es")
                            nc.scalar.activation(
                                es[:], sps[:],
                                mybir.ActivationFunctionType.Exp,
                                scale=scl[:, ds(b * SEXP + 2 * mt + h2, 1)])
                            delta = j * 128 - i * 512
                            if delta >= 0:
                                nc.vector.tensor_mul(
                                    es[:], es[:],
                                    msk[:, ds((delta // 128) * 512, 512)])
                            nc.tensor.matmul(
                                avs[h2][:],
                                vt[:, ds((g * TK + j) * (DH + 1), DH + 1)],
                                es[:], start=(j == 0), stop=(j == ntk - 1))
                    for h2 in range(2):
                        av = avs[h2]
                        rec = rcp.tile([1, 512], F32, tag="rec")
                        nc.vector.reciprocal(rec[:], av[ds(DH, 1), :])
                        recb = rcp.tile([1, 512], BF16, tag="recb")
                        nc.vector.tensor_copy(recb[:], rec[:])
                        bc = bcp.tile([64, 512], F32, tag="bcp")
                        nc.tensor.matmul(bc[:], ones[:], recb[:],
                                         start=True, stop=True)
                        bcs = bsp.tile([64, 512], F32, tag="bcs")
                        nc.scalar.copy(bcs[:], bc[:])
                        nc.vector.tensor_mul(
                            att[ds(h2 * 64, 64), ds(mt * T + i * 512, 512)],
                            av[ds(0, 64), :], bcs[:])

            # ---- o_proj: y = att @ Wo over all 16 heads, fp16 out ----
            for tq in range(TK):
                ysb = ysp.tile([128, D], F16, tag="ysb")
                for dn in range(2):
                    yp = pp.tile([128, 512], F32, tag="pp")
                    for kc in range(KD):
                        nc.tensor.matmul(
                            yp[:],
                            att[:, ds(kc * T + tq * 128, 128)],
                            wo[:, ds(kc * D + dn * 512, 512)],
                            start=(kc == 0), stop=(kc == KD - 1))
                    nc.scalar.activation(ysb[:, ds(dn * 512, 512)], yp[:],
                                         mybir.ActivationFunctionType.Copy,
                                         scale=scl[:, ds(b * SEXP + H, 1)])
                nc.sync.dma_start(out_d[ds(b * T + tq * 128, 128), :],
                                  ysb[:])


_CACHE = {}


def _get_program():
    if "nc" not in _CACHE:
        nc = bacc.Bacc("TRN2", target_bir_lowering=False, debug=False,
                       num_devices=N_CORES)
        with tile.TileContext(nc) as tc:
            _build_body(tc)
        nc.compile()
        _CACHE["nc"] = nc
    return _CACHE["nc"]


def _host_tables():
    freqs = 1.0 / ROPE_THETA ** (np.arange(0, DH, 2, dtype=np.float32) / DH)
    ang = np.outer(np.arange(T, dtype=np.float32), freqs)
    cosT = np.ascontiguousarray(np.cos(ang).T.astype(np.float32))  # [32, T]
    sinT = np.ascontiguousarray(np.sin(ang).T.astype(np.float32))
    cs = np.concatenate([cosT, sinT, cosT, sinT],
                        axis=0).astype(np.float16)  # [128, T]
    return cs


def _quant_i8(w):
    """Symmetric per-tensor int8: returns (int8 array, scale)."""
    s = max(np.abs(w).max() / 127.0, 1e-30)
    return np.clip(np.round(w / s), -127, 127).astype(np.int8), s


def make_in_maps(x, Wq, Wk, Wv, Wo):
    cs = _host_tables()
    eo = np.concatenate([np.arange(0, DH, 2), np.arange(1, DH, 2)])
    qcols = np.concatenate([h * DH + eo for h in range(H)])
    kcols = np.concatenate([g * DH + eo for g in range(KV)])
    wq_p = np.ascontiguousarray(Wq[:, qcols]).astype(np.float32)
    wk_p = Wk[:, kcols].astype(np.float32)
    wv_p = Wv.astype(np.float32)
    wo_p = Wo.astype(np.float32)

    if X_INT8:
        xs, sx = [], np.zeros(B, np.float32)
        for b in range(B):
            xq, s = _quant_i8(x[b].T)
            xs.append(xq)
            sx[b] = s
        xt_h = np.concatenate(xs, axis=0)
    else:
        xt_h = np.concatenate([x[b].T for b in range(B)], axis=0).astype(BF)
        sx = np.ones(B, np.float32)

    if W_INT8:
        sq = np.zeros(H, np.float32)
        wq_h = np.empty_like(wq_p, dtype=np.int8)
        for h in range(H):
            wq_h[:, h * DH:(h + 1) * DH], sq[h] = \
                _quant_i8(wq_p[:, h * DH:(h + 1) * DH])
        sk = np.zeros(KV, np.float32)
        sv = np.zeros(KV, np.float32)
        wkv_h = np.empty((D, 2 * KV * DH), dtype=np.int8)
        for g in range(KV):
            wkv_h[:, g * DH:(g + 1) * DH], sk[g] = \
                _quant_i8(wk_p[:, g * DH:(g + 1) * DH])
            wkv_h[:, KV * DH + g * DH:KV * DH + (g + 1) * DH], sv[g] = \
                _quant_i8(wv_p[:, g * DH:(g + 1) * DH])
    else:
        sq = np.ones(H, np.float32)
        sk = np.ones(KV, np.float32)
        sv = np.ones(KV, np.float32)
        wq_h = wq_p.astype(BF)
        wkv_h = np.concatenate([wk_p, wv_p], axis=1).astype(BF)

    # Wo: fold sx_b * sv_g into rows per head -- but sx varies per batch and
    # Wo is shared across the b-loop, so fold only sv (batch-independent)
    # into Wo and put sx_b into the per-batch output scale column.
    row_scale = np.repeat(sv[np.arange(H) // (H // KV)], DH)
    wo_f = wo_p * row_scale[:, None]
    if W_INT8:
        wo_h, so = _quant_i8(wo_f)
    else:
        wo_h, so = wo_f.astype(BF), 1.0

    scl = np.zeros((128, NB * SEXP), np.float32)
    for b in range(B):
        scl[:, b * SEXP:b * SEXP + H] = \
            SCALE * sx[b] * sx[b] * sq * sk[np.arange(H) // (H // KV)]
        scl[:, b * SEXP + H] = so * sx[b]

    return [{
        "xt": xt_h,
        "wq": wq_h,
        "wkv": wkv_h,
        "wo": wo_h,
        "cs": cs,
        "scl": scl,
    }]


class _Result:
    """Minimal stand-in for BassKernelResults (no NTFF hook available)."""

    def __init__(self, results):
        self.results = results
        self.exec_time_ns = None
        self.mean_exec_time_ns = None
        self.max_exec_time_core_id = None
        self.instructions_and_trace = None
        self.profile_json = None


def _make_launcher(nc, n_cores):
    """Like bass2jax.run_bass_via_pjrt, but the donated output buffers are
    created on-device (jitted zeros) instead of uploaded through the tunnel,
    and the jitted callable is built once and reused."""
    bass2jax.install_neuronx_cc_hook()

    partition_name = (nc.partition_id_tensor.name
                      if nc.partition_id_tensor else None)
    in_names, out_names, out_avals = [], [], []
    for alloc in nc.m.functions[0].allocations:
        if not isinstance(alloc, mybir.MemoryLocationSet):
            continue
        name = alloc.memorylocations[0].name
        if alloc.kind == "ExternalInput":
            if name != partition_name:
                in_names.append(name)
        elif alloc.kind == "ExternalOutput":
            out_names.append(name)
            out_avals.append(jax.core.ShapedArray(
                tuple(alloc.tensor_shape), mybir.dt.np(alloc.dtype)))
    n_params = len(in_names)
    n_outs = len(out_names)
    in_names = in_names + out_names
    if partition_name is not None:
        in_names.append(partition_name)
    donate = tuple(range(n_params, n_params + n_outs))

    def _body(*args):
        operands = list(args)
        if partition_name is not None:
            operands.append(bass2jax.partition_id_tensor())
        outs = bass2jax._bass_exec_p.bind(
            *operands,
            out_avals=tuple(out_avals),
            in_names=tuple(in_names),
            out_names=tuple(out_names),
            lowering_input_output_aliases=(),
            sim_require_finite=True,
            sim_require_nnan=True,
            nc=nc,
        )
        return tuple(outs)

    devices = jax.devices()[:n_cores]
    mesh = bass2jax.Mesh(np.asarray(devices), ("core",))
    pspec = bass2jax.PartitionSpec("core")
    in_specs = (pspec,) * (n_params + n_outs)
    out_specs = (pspec,) * n_outs
    sharded = jax.jit(
        bass2jax.shard_map(_body, mesh=mesh, in_specs=in_specs,
                           out_specs=out_specs, check_rep=False),
        donate_argnums=donate, keep_unused=True)
    shard = jax.sharding.NamedSharding(mesh, pspec)
    zero_fns = [
        jax.jit(lambda a=a: jnp.zeros((n_cores * a.shape[0], *a.shape[1:]),
                                      a.dtype), out_shardings=shard)
        for a in out_avals
    ]

    def launch(in_maps):
        if nc.dbg_addr is not None:
            assert not nc.dbg_callbacks
            in_maps = [
                {**m, nc.dbg_addr.name: np.zeros((1, 2), np.uint32)}
                for m in in_maps
            ]
        concat_in = [
            np.concatenate([np.asarray(in_maps[c][in_names[i]])
                            for c in range(n_cores)], axis=0)
            for i in range(n_params)
        ]
        out_arrs = sharded(*concat_in, *[zf() for zf in zero_fns])
        return [
            {name: np.asarray(out_arrs[i]).reshape(
                n_cores, *out_avals[i].shape)[c]
             for i, name in enumerate(out_names)}
            for c in range(n_cores)
        ]

    return launch


def _run_pjrt(nc, in_maps):
    if "launcher" not in _CACHE:
        _CACHE["launcher"] = _make_launcher(nc, len(in_maps))
    return _CACHE["launcher"](in_maps)


def run(x, Wq, Wk, Wv, Wo, trace=False, tmpdir=None):
    nc = _get_program()
    in_maps = make_in_maps(x, Wq, Wk, Wv, Wo)
    results = _run_pjrt(nc, in_maps)
    out = np.asarray(results[0]["out"]).astype(np.float32).reshape(B, T, D)
    return out, _Result(results)


def kernel(x, mask, Wq, Wk, Wv, Wo):
    x = np.asarray(x, dtype=np.float32)
    out, _ = run(x, np.asarray(Wq, dtype=np.float32),
                 np.asarray(Wk, dtype=np.float32),
                 np.asarray(Wv, dtype=np.float32),
                 np.asarray(Wo, dtype=np.float32))
    return out
